# revision 29
# baseline (speedup 1.0000x reference)
"""Trainium2 Bass kernel for nn_OA_Layer (offset-attention layer).

Reference (per batch b, C=256, N=4096, CQK=64):
    xs = x + xyz
    q = k = wqk @ xs + bqk          [64, N]
    v = wv @ xs + bv                [C, N]
    E = q^T q                       [N, N]  (symmetric, since q == k)
    attn = softmax(E, rows) ; attn /= (1e-9 + attn.sum(rows))
    x_r = v @ attn
    t = wt @ (xs - x_r) + bt ; t = BN(t) ; x_r = leaky_relu(t, 0.2)
    out = xs + x_r

Sharding: data-parallel over batch B=8 across 8 cores (1 batch/core).

Math restructuring (checked on host: rel err ~1.2e-3 vs fp32 reference):
  - all PE operands in bf16 (1 cycle/row vs 2-4 for fp32/f32r)
  - softmax via exp(E - diag - ln(rowsum)): the per-row shift diag[n] =
    ||q_n||^2 prevents overflow and cancels exactly between passes; the
    ln(rowsum) term folds the softmax division into the pass-2 exp bias,
    so a2 tiles come out of the ACT engine already row-normalized.
  - colsum'[m] = sum_n a2[n,m] via a ones-vector matmul accumulated in
    PSUM across the i loop (1 matmul / 512-col tile, not 4 tiny matvecs)
  - x_r = (v' @ a2) * invcs[m]; bv folded into bt' = bt - wt @ bv on host
  - BN+bias folded to t*g + bp_eff on host.
"""

import numpy as np

import concourse.bass as bass
import concourse.tile as tile
from concourse import bacc, mybir
from concourse._compat import with_exitstack

F32 = mybir.dt.float32
F32R = mybir.dt.float32r
BF16 = mybir.dt.bfloat16
FP8 = mybir.dt.float8e4

C = 256
CQK = 64
P = 128
BN_EPS = 1e-5


def build_kernel(N=4096, debug=False):
    """Builds the per-core bass program. Returns nc."""
    nc = bacc.Bacc("TRN2", target_bir_lowering=False, debug=debug,
                   num_devices=8)

    x_d = nc.declare_dram_parameter("x", [C, N], F32, isOutput=False)
    xyz_d = nc.declare_dram_parameter("xyz", [C, N], F32, isOutput=False)
    wqkT_d = nc.declare_dram_parameter("wqkT", [C, CQK], F32, isOutput=False)
    wvT_d = nc.declare_dram_parameter("wvT", [C, C], F32, isOutput=False)
    wtT_d = nc.declare_dram_parameter("wtT", [C, C], F32, isOutput=False)
    bqk_d = nc.declare_dram_parameter("bqk", [CQK, 1], F32, isOutput=False)
    g_d = nc.declare_dram_parameter("g", [C, 1], F32, isOutput=False)
    bp_d = nc.declare_dram_parameter("bp", [C, 1], F32, isOutput=False)
    out_d = nc.declare_dram_parameter("out", [C, N], F32, isOutput=True)

    with tile.TileContext(nc) as tc:
        _emit(nc, tc, N,
              x_d, xyz_d, wqkT_d, wvT_d, wtT_d, bqk_d, g_d, bp_d, out_d)
    nc.compile()
    return nc


@with_exitstack
def _emit(ctx, nc, tc, N,
          x_d, xyz_d, wqkT_d, wvT_d, wtT_d, bqk_d, g_d, bp_d, out_d):
    NB = N // P          # n row-blocks of 128
    MC = N // 512        # m chunks of 512
    ek = ctx.enter_context

    consts = ek(tc.tile_pool(name="consts", bufs=1))
    big = ek(tc.tile_pool(name="big", bufs=1))
    stats = ek(tc.tile_pool(name="stats", bufs=1))

    # ---- constant / resident tensors (fp32 staging -> bf16 for the PE) ----
    wqkT = consts.tile([P, 2 * CQK], F32)       # [p, (khalf, o)]
    nc.sync.dma_start(wqkT[:].rearrange("p (t m) -> p t m", t=2),
                      wqkT_d[:].rearrange("(t p) m -> p t m", p=P))
    wqkT_b = consts.tile([P, 2 * CQK], BF16)
    nc.vector.tensor_copy(wqkT_b[:], wqkT[:])
    bqk = consts.tile([CQK, 1], F32)
    nc.sync.dma_start(bqk[:], bqk_d[:])
    ones64_b = consts.tile([CQK, 1], BF16)
    nc.vector.memset(ones64_b[:], 1.0)
    # two fp8 ones at stride 32 (dual-fp8 ldweights wants even, 16B-aligned
    # outermost weight step)
    ones8 = consts.tile([P, 64], FP8)
    nc.vector.memset(ones8[:], 1.0)
    ones_row = consts.tile([1, P], F32R)
    nc.vector.memset(ones_row[:].bitcast(F32), 1.0)

    # xs = x + xyz in bf16 only, layout [128, 2*N], summed in 1024-col
    # chunks so the q matmuls start as soon as the first chunks land
    xs_b = big.tile([P, 2 * N], BF16)
    zpool = ek(tc.tile_pool(name="zpool", bufs=3))
    XC = 1024
    # x via the tensor queue, xyz via the scalar queue: both engines idle
    # at startup, so the 16 input DMAs trigger in parallel instead of
    # serializing on the sync engine
    for cc in range(N // XC):
        for h in range(2):
            c0 = h * N + cc * XC
            xin = zpool.tile([P, XC], F32, tag="xin")
            nc.scalar.dma_start(xin[:],
                                x_d[h * P:(h + 1) * P, cc * XC:(cc + 1) * XC])
            zin = zpool.tile([P, XC], F32, tag="zin")
            nc.gpsimd.dma_start(
                zin[:], xyz_d[h * P:(h + 1) * P, cc * XC:(cc + 1) * XC])
            nc.vector.tensor_add(xs_b[:, c0:c0 + XC], xin[:], zin[:])

    # weights not needed until pass 1 / the tails — loaded after xs so the
    # first xs chunks (which gate everything) hit the DMA queues first
    wvT = consts.tile([P, 2 * C], F32)
    nc.gpsimd.dma_start(wvT[:].rearrange("p (t m) -> p t m", t=2),
                        wvT_d[:].rearrange("(t p) m -> p t m", p=P))
    wtT = consts.tile([P, 2 * C], F32)
    nc.gpsimd.dma_start(wtT[:].rearrange("p (t m) -> p t m", t=2),
                        wtT_d[:].rearrange("(t p) m -> p t m", p=P))
    wvT_b = consts.tile([P, 2 * C], BF16)
    nc.vector.tensor_copy(wvT_b[:], wvT[:])
    wtT_b = consts.tile([P, 2 * C], BF16)
    nc.vector.tensor_copy(wtT_b[:], wtT[:])
    g_t = consts.tile([P, 2], F32)
    bp_t = consts.tile([P, 2], F32)
    for h in range(2):
        nc.sync.dma_start(g_t[:, h:h + 1], g_d[h * P:(h + 1) * P, :])
        nc.sync.dma_start(bp_t[:, h:h + 1], bp_d[h * P:(h + 1) * P, :])

    # q2: q duplicated on partition halves 0-63 / 64-127 (for PE row-packing)
    q2 = big.tile([P, N], BF16)
    # v^T in fp8 for DoubleRow; col layout (ip, h, u, c2): block i=2*ip+u,
    # channel c = h*128 + c2 lives at col ip*512 + h*256 + u*128 + c2
    vT = big.tile([P, NB * C], FP8)
    # E - diag for j-chunks 0,1, precomputed during pass 1 (PE slack there);
    # bf16 is safe for the shifted values. col = j*(NB*512) + i*512 + m
    Epre = big.tile([P, 2 * NB * 512], BF16)

    # ---- q = wqk @ xs + bqk (bf16) ; diag[n] = ||q_n||^2 ; v^T ----
    diag_row = stats.tile([1, N], F32)
    negdiag = stats.tile([P, NB], F32)
    with tc.tile_pool(name="qvps", bufs=2,
                      space=bass.MemorySpace.PSUM) as qvps:
        # q, its partition-duplicate, diag and negdiag all per 512-chunk so
        # pass 1 can start before the full q is done
        with tc.tile_pool(name="sqp", bufs=2) as sqpool:
            for j in range(MC):
                q_ps = qvps.tile([CQK, 512], F32, tag="q_ps")
                for k in range(2):
                    nc.tensor.matmul(q_ps[:], wqkT_b[:, k * CQK:(k + 1) * CQK],
                                     xs_b[:, k * N + j * 512: k * N + j * 512 + 512],
                                     start=(k == 0), stop=(k == 1))
                nc.vector.tensor_scalar_add(q2[0:CQK, j * 512:(j + 1) * 512],
                                            q_ps[:], bqk[:])
                nc.sync.dma_start(q2[CQK:P, j * 512:(j + 1) * 512],
                                  q2[0:CQK, j * 512:(j + 1) * 512])
                sq = sqpool.tile([CQK, 512], BF16, tag="sq")
                qs = q2[0:CQK, j * 512:(j + 1) * 512]
                nc.vector.tensor_mul(sq[:], qs, qs)
                dg_ps = qvps.tile([1, 512], F32, tag="dg_ps")
                nc.tensor.matmul(dg_ps[:], ones64_b[:], sq[:],
                                 start=True, stop=True)
                nc.vector.tensor_scalar_mul(diag_row[:, j * 512:(j + 1) * 512],
                                            dg_ps[:], -1.0)
                for i in range(4 * j, 4 * j + 4):
                    nc.sync.dma_start(negdiag[:, i:i + 1],
                                      diag_row[0:1, i * P:(i + 1) * P])

    # ---- pass 1: rowsums of exp(E - diag) ----
    SW = min(2048, N)              # strip width
    SPB = N // SW                  # strips per block
    CPS = SW // 512                # 512-chunks per strip
    rs_acc = stats.tile([P, SPB * NB], F32)
    with (
        tc.tile_pool(name="p1ps", bufs=2, space=bass.MemorySpace.PSUM) as p1ps,
        tc.tile_pool(name="p1sc", bufs=2) as p1sc,
    ):
        for i in range(NB):
            for s in range(SPB):
                estrip = p1ps.tile([P, SW], F32, tag="estrip")
                for jj in range(CPS):
                    m0 = s * SW + jj * 512
                    qrow = (CQK if jj % 2 == 1 else 0)
                    nc.tensor.matmul(
                        estrip[:, jj * 512:(jj + 1) * 512],
                        q2[qrow:qrow + CQK, i * P:(i + 1) * P],
                        q2[qrow:qrow + CQK, m0:m0 + 512],
                        start=True, stop=True)
                sink = p1sc.tile([P, SW], F32, tag="sink")
                nc.scalar.activation(
                    sink[:], estrip[:], mybir.ActivationFunctionType.Exp,
                    bias=negdiag[:, i:i + 1],
                    accum_out=rs_acc[:, i * SPB + s: i * SPB + s + 1])
        for i in range(NB):
            ip, u = i // 2, i % 2
            vtile = p1ps.tile([P, SW], F32, tag="estrip", name=f"v_{i}")
            for k in range(2):
                nc.tensor.matmul(vtile[:, 0:C],
                                 xs_b[:, k * N + i * P: k * N + i * P + P],
                                 wvT_b[:, k * C:(k + 1) * C],
                                 start=(k == 0), stop=(k == 1))
            for h in range(2):
                dst = ip * 512 + h * 256 + u * P
                nc.vector.tensor_copy(vT[:, dst:dst + P],
                                      vtile[:, h * P:(h + 1) * P])
            et = p1ps.tile([P, SW], F32, tag="estrip", name=f"ep_{i}")
            for j in range(2):
                qrow = (CQK if j % 2 == 1 else 0)
                nc.tensor.matmul(
                    et[:, j * 512:(j + 1) * 512],
                    q2[qrow:qrow + CQK, i * P:(i + 1) * P],
                    q2[qrow:qrow + CQK, j * 512:(j + 1) * 512],
                    start=True, stop=True)
                nc.vector.tensor_scalar_add(
                    Epre[:, j * (NB * 512) + i * 512:
                         j * (NB * 512) + (i + 1) * 512],
                    et[:, j * 512:(j + 1) * 512], negdiag[:, i:i + 1])

    # bias2 = negdiag - ln(rowsum): pass-2 exp then yields row-normalized attn
    rs_sum = stats.tile([P, NB], F32)
    if SPB == 2:
        nc.vector.tensor_add(rs_sum[:], rs_acc[:, 0:2 * NB:2],
                             rs_acc[:, 1:2 * NB:2])
    else:
        nc.vector.tensor_copy(rs_sum[:], rs_acc[:])
    lnrs = stats.tile([P, NB], F32)
    nc.scalar.activation(lnrs[:], rs_sum[:], mybir.ActivationFunctionType.Ln)
    bias2 = stats.tile([P, NB], F32)
    nc.vector.tensor_sub(bias2[:], negdiag[:], lnrs[:])
    neglnrs = stats.tile([P, NB], F32)
    nc.vector.tensor_scalar_mul(neglnrs[:], lnrs[:], -1.0)

    # ---- pass 2 ----
    with (
        tc.tile_pool(name="e2ps", bufs=3, space=bass.MemorySpace.PSUM) as e2ps,
        tc.tile_pool(name="xrps", bufs=2, space=bass.MemorySpace.PSUM) as xrps,
        tc.tile_pool(name="csps", bufs=1, space=bass.MemorySpace.PSUM) as csps,
        tc.tile_pool(name="a2p", bufs=5) as a2p,
        tc.tile_pool(name="tails", bufs=2) as tails,
    ):
        dbl = mybir.MatmulPerfMode.DoubleRow
        ones8r = ones8[:].rearrange("p (two o) -> p two o", two=2)[:, :, 0:1]
        NP = NB // 2

        tailq = []

        def tail_pre(j, xr, cs_ps):
            # invcs = 1 / (1e-9 + colsum): DVE-only prefix emitted when the
            # colsum stops; the PE suffix is deferred so the in-order PE
            # queue never waits on the reciprocal chain
            cs_eps = tails.tile([1, 512], F32, tag="cs_eps")
            nc.vector.tensor_scalar_add(cs_eps[:], cs_ps[:], 1e-9)
            invcs_f = tails.tile([1, 512], F32, tag="invcs_f")
            nc.vector.reciprocal(invcs_f[:], cs_eps[:])
            invcs_r = tails.tile([1, 512], F32R, tag=f"invcs_r{j % 2}")
            nc.vector.tensor_copy(invcs_r[:], invcs_f[:])
            tailq.append((j, xr, invcs_r))

        def tail_post():
            j, xr, invcs_r = tailq.pop(0)
            bc_ps = e2ps.tile([P, 512], F32, tag="e2strip", name=f"bc_{j}")
            nc.tensor.matmul(bc_ps[:], ones_row[:], invcs_r[:],
                             start=True, stop=True)
            invcs_bc = tails.tile([P, 512], F32, tag="invcs_bc")
            nc.vector.tensor_copy(invcs_bc[:], bc_ps[:])

            # y = xs - x_r * invcs ; t = wt @ y ; BN affine ; lrelu ; + xs
            ys = []
            for h in range(2):
                tmp = tails.tile([P, 512], F32, tag=f"tmp{h}")
                nc.vector.tensor_mul(tmp[:], xr[h][:], invcs_bc[:])
                y_h = tails.tile([P, 512], BF16, tag=f"y{h}")
                nc.vector.tensor_sub(
                    y_h[:], xs_b[:, h * N + j * 512: h * N + j * 512 + 512],
                    tmp[:])
                ys.append(y_h)
            for ho in range(2):
                t_ps = xrps.tile([P, 512], F32, tag=f"xr{ho}",
                                 name=f"tps{ho}_{j}")
                for k in range(2):
                    nc.tensor.matmul(
                        t_ps[:],
                        wtT_b[:, k * C + ho * P: k * C + ho * P + P],
                        ys[k][:], start=(k == 0), stop=(k == 1))
                bn = tails.tile([P, 512], F32, tag=f"bn{ho}")
                nc.vector.tensor_scalar(bn[:], t_ps[:], g_t[:, ho:ho + 1],
                                        bp_t[:, ho:ho + 1],
                                        mybir.AluOpType.mult,
                                        mybir.AluOpType.add)
                lr = tails.tile([P, 512], F32, tag=f"lr{ho}")
                nc.vector.scalar_tensor_tensor(lr[:], bn[:], 0.2, bn[:],
                                               mybir.AluOpType.mult,
                                               mybir.AluOpType.max)
                o_t = tails.tile([P, 512], F32, tag=f"o{ho}")
                nc.vector.tensor_add(
                    o_t[:], lr[:],
                    xs_b[:, ho * N + j * 512: ho * N + j * 512 + 512])
                nc.sync.dma_start(
                    out_d[ho * P:(ho + 1) * P, j * 512:(j + 1) * 512], o_t[:])

        # software-pipelined: xr/cs for (j, ip) are emitted two E/exp pairs
        # later, so the PE never waits on the ACT engine
        pendq = []

        def flush_pend():
            a2r, ip, j, xr, cs_ps = pendq.pop(0)
            first, last = (ip == 0), (ip == NP - 1)
            for h in range(2):
                vsl = vT[:, ip * 512 + h * 256: ip * 512 + (h + 1) * 256]
                nc.tensor.matmul(
                    xr[h][:],
                    vsl.rearrange("p (two c) -> p two c", two=2),
                    a2r,
                    start=first, stop=last, perf_mode=dbl)
            nc.tensor.matmul(cs_ps[:], ones8r, a2r,
                             start=first, stop=last, perf_mode=dbl,
                             skip_group_check=True)
            if last:
                tail_pre(j, xr, cs_ps)

        for j in range(MC):
            cs_ps = csps.tile([1, 512], F32, tag="cs", name=f"cs_{j}")
            xr = [xrps.tile([P, 512], F32, tag=f"xr{h}", name=f"xr{h}_{j}")
                  for h in range(2)]
            for ip in range(NP):
                a2 = a2p.tile([P, 1024], FP8, tag="a2")
                for u in range(2):
                    i = 2 * ip + u
                    if j < 2:
                        nc.scalar.activation(
                            a2[:, u * 512:(u + 1) * 512],
                            Epre[:, j * (NB * 512) + i * 512:
                                 j * (NB * 512) + (i + 1) * 512],
                            mybir.ActivationFunctionType.Exp,
                            bias=neglnrs[:, i:i + 1])
                        continue
                    estrip = e2ps.tile([P, 512], F32, tag="e2strip")
                    qrow = (CQK if u == 1 else 0)
                    nc.tensor.matmul(
                        estrip[:],
                        q2[qrow:qrow + CQK, i * P:(i + 1) * P],
                        q2[qrow:qrow + CQK, j * 512:(j + 1) * 512],
                        start=True, stop=True)
                    nc.scalar.activation(a2[:, u * 512:(u + 1) * 512],
                                         estrip[:],
                                         mybir.ActivationFunctionType.Exp,
                                         bias=bias2[:, i:i + 1])
                if len(pendq) >= 3:
                    flush_pend()
                if tailq and ip == 6:
                    tail_post()
                pendq.append((a2[:].rearrange("p (two n) -> p two n", two=2),
                              ip, j, xr, cs_ps))
        while pendq:
            flush_pend()
        while tailq:
            tail_post()


# ---------------------------------------------------------------------------
# host-side wrapper
# ---------------------------------------------------------------------------
_NC_CACHE = {}


def _get_nc(N=4096):
    if N not in _NC_CACHE:
        _NC_CACHE[N] = build_kernel(N=N)
    return _NC_CACHE[N]


def host_prep(wqk, bqk, wv, bv, wt, bt, bn_gamma, bn_beta, bn_mean, bn_var):
    wqk = np.asarray(wqk, np.float32)
    wv = np.asarray(wv, np.float32)
    wt = np.asarray(wt, np.float32)
    g = (np.asarray(bn_gamma, np.float32)
         / np.sqrt(np.asarray(bn_var, np.float32) + BN_EPS))
    bp = np.asarray(bn_beta, np.float32) - np.asarray(bn_mean, np.float32) * g
    btp = np.asarray(bt, np.float32) - wt @ np.asarray(bv, np.float32)
    bp_eff = btp * g + bp
    return {
        "wqkT": np.ascontiguousarray(wqk.T),
        "wvT": np.ascontiguousarray(wv.T),
        "wtT": np.ascontiguousarray(wt.T),
        "bqk": np.asarray(bqk, np.float32).reshape(CQK, 1),
        "g": g.reshape(C, 1),
        "bp": bp_eff.reshape(C, 1),
    }


def kernel(x, xyz, wqk, bqk, wv, bv, wt, bt, bn_gamma, bn_beta, bn_mean,
           bn_var, _profile=False):
    from concourse.bass_utils import run_bass_kernel_spmd

    x = np.asarray(x, np.float32)
    xyz = np.asarray(xyz, np.float32)
    B, Cc, N = x.shape
    assert Cc == C and B == 8
    nc = _get_nc(N)
    wmap = host_prep(wqk, bqk, wv, bv, wt, bt, bn_gamma, bn_beta, bn_mean,
                     bn_var)
    in_maps = [
        {"x": np.ascontiguousarray(x[b]),
         "xyz": np.ascontiguousarray(xyz[b]), **wmap}
        for b in range(B)
    ]
    res = run_bass_kernel_spmd(nc, in_maps, list(range(8)), trace=_profile)
    out = np.stack([res.results[b]["out"] for b in range(B)], axis=0)
    if _profile:
        return out, res
    return out


# revision 30
# speedup vs baseline: 1.0144x; 1.0144x over previous
"""Trainium2 Bass kernel for nn_OA_Layer (offset-attention layer).

Reference (per batch b, C=256, N=4096, CQK=64):
    xs = x + xyz
    q = k = wqk @ xs + bqk          [64, N]
    v = wv @ xs + bv                [C, N]
    E = q^T q                       [N, N]  (symmetric, since q == k)
    attn = softmax(E, rows) ; attn /= (1e-9 + attn.sum(rows))
    x_r = v @ attn
    t = wt @ (xs - x_r) + bt ; t = BN(t) ; x_r = leaky_relu(t, 0.2)
    out = xs + x_r

Sharding: data-parallel over batch B=8 across 8 cores (1 batch/core).

Math restructuring (checked on host: rel err ~1.2e-3 vs fp32 reference):
  - all PE operands in bf16 (1 cycle/row vs 2-4 for fp32/f32r)
  - softmax via exp(E - diag - ln(rowsum)): the per-row shift diag[n] =
    ||q_n||^2 prevents overflow and cancels exactly between passes; the
    ln(rowsum) term folds the softmax division into the pass-2 exp bias,
    so a2 tiles come out of the ACT engine already row-normalized.
  - colsum'[m] = sum_n a2[n,m] via a ones-vector matmul accumulated in
    PSUM across the i loop (1 matmul / 512-col tile, not 4 tiny matvecs)
  - x_r = (v' @ a2) * invcs[m]; bv folded into bt' = bt - wt @ bv on host
  - BN+bias folded to t*g + bp_eff on host.
"""

import numpy as np

import concourse.bass as bass
import concourse.tile as tile
from concourse import bacc, mybir
from concourse._compat import with_exitstack

F32 = mybir.dt.float32
F32R = mybir.dt.float32r
BF16 = mybir.dt.bfloat16
FP8 = mybir.dt.float8e4

C = 256
CQK = 64
P = 128
BN_EPS = 1e-5


def build_kernel(N=4096, debug=False):
    """Builds the per-core bass program. Returns nc."""
    nc = bacc.Bacc("TRN2", target_bir_lowering=False, debug=debug,
                   num_devices=8)

    x_d = nc.declare_dram_parameter("x", [C, N], F32, isOutput=False)
    xyz_d = nc.declare_dram_parameter("xyz", [C, N], F32, isOutput=False)
    wqkT_d = nc.declare_dram_parameter("wqkT", [C, CQK], F32, isOutput=False)
    wvT_d = nc.declare_dram_parameter("wvT", [C, C], F32, isOutput=False)
    wtT_d = nc.declare_dram_parameter("wtT", [C, C], F32, isOutput=False)
    bqk_d = nc.declare_dram_parameter("bqk", [CQK, 1], F32, isOutput=False)
    g_d = nc.declare_dram_parameter("g", [C, 1], F32, isOutput=False)
    bp_d = nc.declare_dram_parameter("bp", [C, 1], F32, isOutput=False)
    out_d = nc.declare_dram_parameter("out", [C, N], F32, isOutput=True)

    with tile.TileContext(nc) as tc:
        _emit(nc, tc, N,
              x_d, xyz_d, wqkT_d, wvT_d, wtT_d, bqk_d, g_d, bp_d, out_d)
    nc.compile()
    return nc


@with_exitstack
def _emit(ctx, nc, tc, N,
          x_d, xyz_d, wqkT_d, wvT_d, wtT_d, bqk_d, g_d, bp_d, out_d):
    NB = N // P          # n row-blocks of 128
    MC = N // 512        # m chunks of 512
    ek = ctx.enter_context

    consts = ek(tc.tile_pool(name="consts", bufs=1))
    big = ek(tc.tile_pool(name="big", bufs=1))
    stats = ek(tc.tile_pool(name="stats", bufs=1))

    # ---- constant / resident tensors (fp32 staging -> bf16 for the PE) ----
    wqkT = consts.tile([P, 2 * CQK], F32)       # [p, (khalf, o)]
    nc.sync.dma_start(wqkT[:].rearrange("p (t m) -> p t m", t=2),
                      wqkT_d[:].rearrange("(t p) m -> p t m", p=P))
    wqkT_b = consts.tile([P, 2 * CQK], BF16)
    nc.vector.tensor_copy(wqkT_b[:], wqkT[:])
    bqk = consts.tile([CQK, 1], F32)
    nc.sync.dma_start(bqk[:], bqk_d[:])
    ones64_b = consts.tile([CQK, 1], BF16)
    nc.vector.memset(ones64_b[:], 1.0)
    # two fp8 ones at stride 32 (dual-fp8 ldweights wants even, 16B-aligned
    # outermost weight step)
    ones8 = consts.tile([P, 64], FP8)
    nc.vector.memset(ones8[:], 1.0)
    ones_row = consts.tile([1, P], F32R)
    nc.vector.memset(ones_row[:].bitcast(F32), 1.0)

    # xs = x + xyz in bf16 only, layout [128, 2*N], summed in 1024-col
    # chunks so the q matmuls start as soon as the first chunks land
    xs_b = big.tile([P, 2 * N], BF16)
    zpool = ek(tc.tile_pool(name="zpool", bufs=3))
    XC = 1024
    # x via the tensor queue, xyz via the scalar queue: both engines idle
    # at startup, so the 16 input DMAs trigger in parallel instead of
    # serializing on the sync engine
    for cc in range(N // XC):
        for h in range(2):
            c0 = h * N + cc * XC
            xin = zpool.tile([P, XC], F32, tag="xin")
            nc.scalar.dma_start(xin[:],
                                x_d[h * P:(h + 1) * P, cc * XC:(cc + 1) * XC])
            zin = zpool.tile([P, XC], F32, tag="zin")
            nc.gpsimd.dma_start(
                zin[:], xyz_d[h * P:(h + 1) * P, cc * XC:(cc + 1) * XC])
            nc.vector.tensor_add(xs_b[:, c0:c0 + XC], xin[:], zin[:])

    # weights not needed until pass 1 / the tails — loaded after xs so the
    # first xs chunks (which gate everything) hit the DMA queues first
    wvT = consts.tile([P, 2 * C], F32)
    nc.gpsimd.dma_start(wvT[:].rearrange("p (t m) -> p t m", t=2),
                        wvT_d[:].rearrange("(t p) m -> p t m", p=P))
    wtT = consts.tile([P, 2 * C], F32)
    nc.gpsimd.dma_start(wtT[:].rearrange("p (t m) -> p t m", t=2),
                        wtT_d[:].rearrange("(t p) m -> p t m", p=P))
    wvT_b = consts.tile([P, 2 * C], BF16)
    nc.vector.tensor_copy(wvT_b[:], wvT[:])
    wtT_b = consts.tile([P, 2 * C], BF16)
    nc.vector.tensor_copy(wtT_b[:], wtT[:])
    g_t = consts.tile([P, 2], F32)
    bp_t = consts.tile([P, 2], F32)
    for h in range(2):
        nc.sync.dma_start(g_t[:, h:h + 1], g_d[h * P:(h + 1) * P, :])
        nc.sync.dma_start(bp_t[:, h:h + 1], bp_d[h * P:(h + 1) * P, :])

    # q2: q duplicated on partition halves 0-63 / 64-127 (for PE row-packing)
    q2 = big.tile([P, N], BF16)
    # v^T in fp8 for DoubleRow; col layout (ip, h, u, c2): block i=2*ip+u,
    # channel c = h*128 + c2 lives at col ip*512 + h*256 + u*128 + c2
    vT = big.tile([P, NB * C], FP8)
    # E - diag for j-chunks 0,1, precomputed during pass 1 (PE slack there);
    # bf16 is safe for the shifted values. col = j*(NB*512) + i*512 + m
    Epre = big.tile([P, 2 * NB * 512], BF16)

    # ---- q/diag per 512-chunk, interleaved with early pass-1 strips so
    # the ACT engine starts exp while the rest of xs is still streaming ----
    diag_row = stats.tile([1, N], F32)
    negdiag = stats.tile([P, NB], F32)
    rs_acc = stats.tile([P, 4 * NB], F32)
    nc.vector.memset(rs_acc[:], 0.0)

    def q_chunk(j, qvps, dgp, sqpool):
        q_ps = qvps.tile([CQK, 512], F32, tag="q_ps")
        for k in range(2):
            nc.tensor.matmul(q_ps[:], wqkT_b[:, k * CQK:(k + 1) * CQK],
                             xs_b[:, k * N + j * 512: k * N + j * 512 + 512],
                             start=(k == 0), stop=(k == 1))
        nc.vector.tensor_scalar_add(q2[0:CQK, j * 512:(j + 1) * 512],
                                    q_ps[:], bqk[:])
        nc.sync.dma_start(q2[CQK:P, j * 512:(j + 1) * 512],
                          q2[0:CQK, j * 512:(j + 1) * 512])
        sq = sqpool.tile([CQK, 512], BF16, tag="sq")
        qs = q2[0:CQK, j * 512:(j + 1) * 512]
        nc.vector.tensor_mul(sq[:], qs, qs)
        dg_ps = dgp.tile([1, 512], F32, tag="dg_ps")
        nc.tensor.matmul(dg_ps[:], ones64_b[:], sq[:], start=True, stop=True)
        nc.vector.tensor_scalar_mul(diag_row[:, j * 512:(j + 1) * 512],
                                    dg_ps[:], -1.0)
        for i in range(4 * j, 4 * j + 4):
            nc.sync.dma_start(negdiag[:, i:i + 1],
                              diag_row[0:1, i * P:(i + 1) * P])

    with (
        tc.tile_pool(name="qp", bufs=2, space=bass.MemorySpace.PSUM) as qvps,
        tc.tile_pool(name="dgp", bufs=1, space=bass.MemorySpace.PSUM) as dgp,
        tc.tile_pool(name="p1a", bufs=2, space=bass.MemorySpace.PSUM) as p1a,
        tc.tile_pool(name="sqp", bufs=2) as sqpool,
        tc.tile_pool(name="p1sca", bufs=2) as p1sca,
    ):
        for j in range(4):
            q_chunk(j, qvps, dgp, sqpool)
        # blocks 0-15 over m in [0, 2048): everything they need is already
        # resident, so exp starts ~25us in while xs cc>=2 still streams
        for i in range(16):
            for s in range(2):
                estrip = p1a.tile([P, 1024], F32, tag="estrip")
                for jj in range(2):
                    m0 = s * 1024 + jj * 512
                    qrow = (CQK if jj % 2 == 1 else 0)
                    nc.tensor.matmul(
                        estrip[:, jj * 512:(jj + 1) * 512],
                        q2[qrow:qrow + CQK, i * P:(i + 1) * P],
                        q2[qrow:qrow + CQK, m0:m0 + 512],
                        start=True, stop=True)
                sink = p1sca.tile([P, 1024], F32, tag="sink")
                nc.scalar.activation(
                    sink[:], estrip[:], mybir.ActivationFunctionType.Exp,
                    bias=negdiag[:, i:i + 1],
                    accum_out=rs_acc[:, 4 * i + s: 4 * i + s + 1])
        for j in range(4, MC):
            q_chunk(j, qvps, dgp, sqpool)

    # ---- pass 1 rest: remaining strips at SW=2048, then v^T and the
    # E-precompute for j-chunks 0,1 (PE slack while ACT drains exps) ----
    with (
        tc.tile_pool(name="p1ps", bufs=2, space=bass.MemorySpace.PSUM) as p1ps,
        tc.tile_pool(name="p1sc", bufs=2) as p1sc,
    ):
        def strip2048(i, m_base, acc_col):
            estrip = p1ps.tile([P, 2048], F32, tag="estrip")
            for jj in range(4):
                m0 = m_base + jj * 512
                qrow = (CQK if jj % 2 == 1 else 0)
                nc.tensor.matmul(
                    estrip[:, jj * 512:(jj + 1) * 512],
                    q2[qrow:qrow + CQK, i * P:(i + 1) * P],
                    q2[qrow:qrow + CQK, m0:m0 + 512],
                    start=True, stop=True)
            sink = p1sc.tile([P, 2048], F32, tag="sink")
            nc.scalar.activation(
                sink[:], estrip[:], mybir.ActivationFunctionType.Exp,
                bias=negdiag[:, i:i + 1],
                accum_out=rs_acc[:, acc_col:acc_col + 1])

        for i in range(16):
            strip2048(i, 2048, 4 * i + 2)
        for i in range(16, NB):
            for s in range(2):
                strip2048(i, s * 2048, 4 * i + s)
        for i in range(NB):
            ip, u = i // 2, i % 2
            vtile = p1ps.tile([P, 2048], F32, tag="estrip", name=f"v_{i}")
            for k in range(2):
                nc.tensor.matmul(vtile[:, 0:C],
                                 xs_b[:, k * N + i * P: k * N + i * P + P],
                                 wvT_b[:, k * C:(k + 1) * C],
                                 start=(k == 0), stop=(k == 1))
            for h in range(2):
                dst = ip * 512 + h * 256 + u * P
                nc.vector.tensor_copy(vT[:, dst:dst + P],
                                      vtile[:, h * P:(h + 1) * P])
            et = p1ps.tile([P, 2048], F32, tag="estrip", name=f"ep_{i}")
            for j in range(2):
                qrow = (CQK if j % 2 == 1 else 0)
                nc.tensor.matmul(
                    et[:, j * 512:(j + 1) * 512],
                    q2[qrow:qrow + CQK, i * P:(i + 1) * P],
                    q2[qrow:qrow + CQK, j * 512:(j + 1) * 512],
                    start=True, stop=True)
                nc.vector.tensor_scalar_add(
                    Epre[:, j * (NB * 512) + i * 512:
                         j * (NB * 512) + (i + 1) * 512],
                    et[:, j * 512:(j + 1) * 512], negdiag[:, i:i + 1])

    # bias2 = negdiag - ln(rowsum): pass-2 exp then yields row-normalized attn
    rs_sum = stats.tile([P, NB], F32)
    rt0 = stats.tile([P, NB], F32)
    rt1 = stats.tile([P, NB], F32)
    nc.vector.tensor_add(rt0[:], rs_acc[:, 0::4], rs_acc[:, 1::4])
    nc.vector.tensor_add(rt1[:], rs_acc[:, 2::4], rs_acc[:, 3::4])
    nc.vector.tensor_add(rs_sum[:], rt0[:], rt1[:])
    lnrs = stats.tile([P, NB], F32)
    nc.scalar.activation(lnrs[:], rs_sum[:], mybir.ActivationFunctionType.Ln)
    bias2 = stats.tile([P, NB], F32)
    nc.vector.tensor_sub(bias2[:], negdiag[:], lnrs[:])
    neglnrs = stats.tile([P, NB], F32)
    nc.vector.tensor_scalar_mul(neglnrs[:], lnrs[:], -1.0)

    # ---- pass 2 ----
    with (
        tc.tile_pool(name="e2ps", bufs=3, space=bass.MemorySpace.PSUM) as e2ps,
        tc.tile_pool(name="xrps", bufs=2, space=bass.MemorySpace.PSUM) as xrps,
        tc.tile_pool(name="csps", bufs=1, space=bass.MemorySpace.PSUM) as csps,
        tc.tile_pool(name="a2p", bufs=5) as a2p,
        tc.tile_pool(name="tails", bufs=2) as tails,
    ):
        dbl = mybir.MatmulPerfMode.DoubleRow
        ones8r = ones8[:].rearrange("p (two o) -> p two o", two=2)[:, :, 0:1]
        NP = NB // 2

        tailq = []

        def tail_pre(j, xr, cs_ps):
            # invcs = 1 / (1e-9 + colsum): DVE-only prefix emitted when the
            # colsum stops; the PE suffix is deferred so the in-order PE
            # queue never waits on the reciprocal chain
            cs_eps = tails.tile([1, 512], F32, tag="cs_eps")
            nc.vector.tensor_scalar_add(cs_eps[:], cs_ps[:], 1e-9)
            invcs_f = tails.tile([1, 512], F32, tag="invcs_f")
            nc.vector.reciprocal(invcs_f[:], cs_eps[:])
            invcs_r = tails.tile([1, 512], F32R, tag=f"invcs_r{j % 2}")
            nc.vector.tensor_copy(invcs_r[:], invcs_f[:])
            tailq.append((j, xr, invcs_r))

        def tail_post():
            j, xr, invcs_r = tailq.pop(0)
            bc_ps = e2ps.tile([P, 512], F32, tag="e2strip", name=f"bc_{j}")
            nc.tensor.matmul(bc_ps[:], ones_row[:], invcs_r[:],
                             start=True, stop=True)
            invcs_bc = tails.tile([P, 512], F32, tag="invcs_bc")
            nc.vector.tensor_copy(invcs_bc[:], bc_ps[:])

            # y = xs - x_r * invcs ; t = wt @ y ; BN affine ; lrelu ; + xs
            ys = []
            for h in range(2):
                tmp = tails.tile([P, 512], F32, tag=f"tmp{h}")
                nc.vector.tensor_mul(tmp[:], xr[h][:], invcs_bc[:])
                y_h = tails.tile([P, 512], BF16, tag=f"y{h}")
                nc.vector.tensor_sub(
                    y_h[:], xs_b[:, h * N + j * 512: h * N + j * 512 + 512],
                    tmp[:])
                ys.append(y_h)
            for ho in range(2):
                t_ps = xrps.tile([P, 512], F32, tag=f"xr{ho}",
                                 name=f"tps{ho}_{j}")
                for k in range(2):
                    nc.tensor.matmul(
                        t_ps[:],
                        wtT_b[:, k * C + ho * P: k * C + ho * P + P],
                        ys[k][:], start=(k == 0), stop=(k == 1))
                bn = tails.tile([P, 512], F32, tag=f"bn{ho}")
                nc.vector.tensor_scalar(bn[:], t_ps[:], g_t[:, ho:ho + 1],
                                        bp_t[:, ho:ho + 1],
                                        mybir.AluOpType.mult,
                                        mybir.AluOpType.add)
                lr = tails.tile([P, 512], F32, tag=f"lr{ho}")
                nc.vector.scalar_tensor_tensor(lr[:], bn[:], 0.2, bn[:],
                                               mybir.AluOpType.mult,
                                               mybir.AluOpType.max)
                o_t = tails.tile([P, 512], F32, tag=f"o{ho}")
                nc.vector.tensor_add(
                    o_t[:], lr[:],
                    xs_b[:, ho * N + j * 512: ho * N + j * 512 + 512])
                nc.sync.dma_start(
                    out_d[ho * P:(ho + 1) * P, j * 512:(j + 1) * 512], o_t[:])

        # software-pipelined: xr/cs for (j, ip) are emitted two E/exp pairs
        # later, so the PE never waits on the ACT engine
        pendq = []

        def flush_pend():
            a2r, ip, j, xr, cs_ps = pendq.pop(0)
            first, last = (ip == 0), (ip == NP - 1)
            for h in range(2):
                vsl = vT[:, ip * 512 + h * 256: ip * 512 + (h + 1) * 256]
                nc.tensor.matmul(
                    xr[h][:],
                    vsl.rearrange("p (two c) -> p two c", two=2),
                    a2r,
                    start=first, stop=last, perf_mode=dbl)
            nc.tensor.matmul(cs_ps[:], ones8r, a2r,
                             start=first, stop=last, perf_mode=dbl,
                             skip_group_check=True)
            if last:
                tail_pre(j, xr, cs_ps)

        for j in range(MC):
            cs_ps = csps.tile([1, 512], F32, tag="cs", name=f"cs_{j}")
            xr = [xrps.tile([P, 512], F32, tag=f"xr{h}", name=f"xr{h}_{j}")
                  for h in range(2)]
            for ip in range(NP):
                a2 = a2p.tile([P, 1024], FP8, tag="a2")
                for u in range(2):
                    i = 2 * ip + u
                    if j < 2:
                        nc.scalar.activation(
                            a2[:, u * 512:(u + 1) * 512],
                            Epre[:, j * (NB * 512) + i * 512:
                                 j * (NB * 512) + (i + 1) * 512],
                            mybir.ActivationFunctionType.Exp,
                            bias=neglnrs[:, i:i + 1])
                        continue
                    estrip = e2ps.tile([P, 512], F32, tag="e2strip")
                    qrow = (CQK if u == 1 else 0)
                    nc.tensor.matmul(
                        estrip[:],
                        q2[qrow:qrow + CQK, i * P:(i + 1) * P],
                        q2[qrow:qrow + CQK, j * 512:(j + 1) * 512],
                        start=True, stop=True)
                    nc.scalar.activation(a2[:, u * 512:(u + 1) * 512],
                                         estrip[:],
                                         mybir.ActivationFunctionType.Exp,
                                         bias=bias2[:, i:i + 1])
                if len(pendq) >= 3:
                    flush_pend()
                if tailq and ip == 6:
                    tail_post()
                pendq.append((a2[:].rearrange("p (two n) -> p two n", two=2),
                              ip, j, xr, cs_ps))
        while pendq:
            flush_pend()
        while tailq:
            tail_post()


# ---------------------------------------------------------------------------
# host-side wrapper
# ---------------------------------------------------------------------------
_NC_CACHE = {}


def _get_nc(N=4096):
    if N not in _NC_CACHE:
        _NC_CACHE[N] = build_kernel(N=N)
    return _NC_CACHE[N]


def host_prep(wqk, bqk, wv, bv, wt, bt, bn_gamma, bn_beta, bn_mean, bn_var):
    wqk = np.asarray(wqk, np.float32)
    wv = np.asarray(wv, np.float32)
    wt = np.asarray(wt, np.float32)
    g = (np.asarray(bn_gamma, np.float32)
         / np.sqrt(np.asarray(bn_var, np.float32) + BN_EPS))
    bp = np.asarray(bn_beta, np.float32) - np.asarray(bn_mean, np.float32) * g
    btp = np.asarray(bt, np.float32) - wt @ np.asarray(bv, np.float32)
    bp_eff = btp * g + bp
    return {
        "wqkT": np.ascontiguousarray(wqk.T),
        "wvT": np.ascontiguousarray(wv.T),
        "wtT": np.ascontiguousarray(wt.T),
        "bqk": np.asarray(bqk, np.float32).reshape(CQK, 1),
        "g": g.reshape(C, 1),
        "bp": bp_eff.reshape(C, 1),
    }


def kernel(x, xyz, wqk, bqk, wv, bv, wt, bt, bn_gamma, bn_beta, bn_mean,
           bn_var, _profile=False):
    from concourse.bass_utils import run_bass_kernel_spmd

    x = np.asarray(x, np.float32)
    xyz = np.asarray(xyz, np.float32)
    B, Cc, N = x.shape
    assert Cc == C and B == 8
    nc = _get_nc(N)
    wmap = host_prep(wqk, bqk, wv, bv, wt, bt, bn_gamma, bn_beta, bn_mean,
                     bn_var)
    in_maps = [
        {"x": np.ascontiguousarray(x[b]),
         "xyz": np.ascontiguousarray(xyz[b]), **wmap}
        for b in range(B)
    ]
    res = run_bass_kernel_spmd(nc, in_maps, list(range(8)), trace=_profile)
    out = np.stack([res.results[b]["out"] for b in range(B)], axis=0)
    if _profile:
        return out, res
    return out
